# revision 2
# baseline (speedup 1.0000x reference)
"""Trainium2 Bass kernel for nn_BalNoisedTopK (balanced noised top-k loss).

loss_i = relu(1 + E_Z[5th-max(s_i^{\\y_i} + Z)] - s_{i,y_i}); out = mean_i.
Pure data parallel over batch: 8 rows/core on 8 cores; host does masking,
dtype/layout prep and the final hinge+means; device does adds + 5th-max.

Key design (measured on this axon-tunneled TRN2 setup):
  - Z is sent as fp8(e4m3) in a partition-outermost layout [P, NI, M, KJ]
    so each partition's slice per DMA is one contiguous run (DMA here costs
    ~78ns/descriptor + bytes/193GB/s; fp8 halves the bytes; rel-err impact
    validated at 2.5e-3 vs the 2e-2 budget).
  - pert = Z + s via mixed-dtype tensor_add (fp8 in0 + fp16 in1 -> fp16),
    split DVE / GPSIMD per noise sample m (GPS_ADD m's on GPSIMD).
  - Two tournament folds (tensor_max of halves, fp16 2x) compress 800 ->
    200 per partition before the 1x Max8 top-8 scan.  A global top-5
    element of rank j sits at local rank <= j of its folded chunk, so the
    per-(i,m) 5th max survives exactly (fold collisions ~1e-4/(i,m),
    measured zero extra error on the real inputs).
  - Stage-2: PE transpose + cross-partition Max8 + DRAM restage + Max8
    gives the exact 5th max of the 1000 candidates per (i,m).
"""

import os
import sys

import numpy as np

for _p in ("/opt/trn_rl_repo", os.path.expanduser("~/.axon_site/_ro/trn_rl_repo")):
    if os.path.isdir(_p) and _p not in sys.path:
        sys.path.insert(0, _p)

N, D, M, K = 64, 100000, 8, 5
NCORES = 8
NI = N // NCORES          # 8 batch rows per core
P = 125                   # SBUF partitions carrying d-chunks
KJ = D // P               # 800 f's per partition
NEG16 = -60000.0
_CACHE = {}

# tuning knobs
FOLD_DEPTH = 2            # folds before Max8 (800 -> 800/2^depth)
GPS_ADD = 4               # how many m's GPSIMD handles for the add
GPS_FOLD1 = 0             # Pool TT-max unsupported by this walrus build
GPS_FOLD2 = 0
ZROWS = 2                 # batch rows per Z DMA
ZBUFS = 4                 # z tile pool buffers (in units of ZROWS rows)
ZDT = "f8mix"             # f16 | f8 (SWDGE cast) | f8act (ACT upconvert)
                          # | f8mix (DVE mixed-dtype adds read fp8 directly)
ZDMA = "gpsimd"           # engine for Z DMA: sync (HWDGE) or gpsimd (SWDGE)
SDT = "f16"               # s HBM dtype: f16, or f8 (ACT upconverts once/iter)


def _split_waits(nc, max_waits=1):
    import concourse.mybir as mybir

    for blk in nc.m.functions[0].blocks:
        new_list = []
        for inst in blk.instructions:
            si = inst.sync_info
            if si is not None and len(si.on_wait) > max_waits:
                waits = list(si.on_wait)
                keep = [w for w in waits if w.wait_reg is not None]
                movable = [w for w in waits if w.wait_reg is None]
                while len(keep) < max_waits and movable:
                    keep.append(movable.pop())
                k = 0
                while movable:
                    chunk, movable = movable[:max_waits], movable[max_waits:]
                    ev = mybir.InstEventSemaphore(
                        name=f"{inst.name}_xw{k}", ins=[], outs=[]
                    )
                    ev.engine = inst.engine
                    ev.sync_info = mybir.SyncInfo(on_wait=chunk, on_update=[])
                    new_list.append(ev)
                    k += 1
                inst.sync_info = mybir.SyncInfo(
                    on_wait=keep, on_update=list(si.on_update)
                )
            new_list.append(inst)
        blk.instructions = new_list
    return nc


def _build_nc(reps=1, split=True, mode="full", loop_reps=0,
              fold_depth=None, gps_add=None, gps_fold1=None, gps_fold2=None,
              zrows=None, zbufs=None, zdt=None, zdma=None, sdt=None):
    import concourse.bass as bass
    import concourse.mybir as mybir
    from concourse.tile import TileContext

    fold_depth = FOLD_DEPTH if fold_depth is None else fold_depth
    gps_add = GPS_ADD if gps_add is None else gps_add
    gps_fold1 = GPS_FOLD1 if gps_fold1 is None else gps_fold1
    gps_fold2 = GPS_FOLD2 if gps_fold2 is None else gps_fold2
    zrows = ZROWS if zrows is None else zrows
    zbufs = ZBUFS if zbufs is None else zbufs
    zdt = ZDT if zdt is None else zdt
    zdma = ZDMA if zdma is None else zdma
    sdt = SDT if sdt is None else sdt

    dt = mybir.dt.float16
    zdram_dt = mybir.dt.float16 if zdt == "f16" else mybir.dt.float8e4
    zsb_dt = dt if zdt in ("f16", "f8") else mybir.dt.float8e4
    nc = bass.Bass("TRN2")
    # z layout: host-prepped [P, NI, M, KJ] (partition-outermost:
    # each partition's (i, m, f) range is one contiguous DRAM run)
    if sdt == "f16":
        s = nc.dram_tensor("s", (NI, D), dt, kind="ExternalInput")
    else:
        s = nc.dram_tensor("s", (P, NI * KJ), mybir.dt.float8e4,
                           kind="ExternalInput")
    z = nc.dram_tensor("z", (P, NI, M, KJ), zdram_dt, kind="ExternalInput")
    ident = nc.dram_tensor("ident", (128, 128), dt, kind="ExternalInput")
    out = nc.dram_tensor("out", (NI, M), dt, kind="ExternalOutput")

    fsz = KJ >> fold_depth      # Max8 input size per (m, partition)

    with TileContext(nc) as tc:
        with (
            tc.tile_pool(name="zpool", bufs=zbufs) as zpool,
            tc.tile_pool(name="fpool", bufs=4) as fpool,
            tc.tile_pool(name="cpool", bufs=1) as cpool,
            tc.tile_pool(name="wpool", bufs=2) as wpool,
            tc.tile_pool(name="ppool", bufs=2, space="PSUM") as ppool,
            tc.tile_pool(name="dpool", bufs=1, space="DRAM") as dpool,
        ):
            identsb = cpool.tile([128, 128], dt)
            nc.sync.dma_start(identsb[:], ident.ap())

            import contextlib

            loop_cm = (
                tc.For_i(0, loop_reps, 1)
                if loop_reps > 0
                else contextlib.nullcontext()
            )
            with loop_cm:
              for _rep in range(reps):
                # all rows' s in one DMA: st_all[p, i*KJ+f] <- s[i, p*KJ+f]
                st_all = wpool.tile([P, NI * KJ], dt, tag="st_all")
                if sdt == "f16":
                    s_src = s.ap().rearrange("i (p f) -> p i f", p=P)
                    nc.sync.dma_start(
                        st_all[:].rearrange("p (i f) -> p i f", i=NI), s_src
                    )
                else:
                    st8 = wpool.tile([P, NI * KJ], mybir.dt.float8e4,
                                     tag="st8")
                    nc.sync.dma_start(st8[:], s.ap())
                    nc.scalar.copy(st_all[:], st8[:])
                cand = wpool.tile([128, NI * M * 8], dt, tag="cand")
                nc.gpsimd.memset(cand[:], NEG16)
                out8 = wpool.tile([64, NI * 8], dt, tag="out8")

                for i0 in range(0, NI, zrows):
                    zt = zpool.tile([P, zrows * M * KJ], zsb_dt, tag="zt")
                    if mode == "compute":
                        nc.gpsimd.memset(zt[:], 0.0)
                    if mode != "compute":
                        # one DMA for zrows rows; contiguous per partition
                        zsrc = z.ap()[:, i0 : i0 + zrows]
                        zdst = zt[:].rearrange(
                            "p (i m f) -> p i m f", i=zrows, m=M
                        )
                        if zdt == "f8" or (zdt == "f16" and zdma == "gpsimd"):
                            # SWDGE path (casts fp8 -> fp16 inline for f8)
                            nc.gpsimd.dma_start(zdst, zsrc)
                        else:
                            nc.sync.dma_start(zdst, zsrc)
                    if mode == "dma":
                        continue
                    for di in range(zrows):
                        i = i0 + di
                        st = st_all[:, i * KJ : (i + 1) * KJ]
                        zrow = zt[:, di * M * KJ : (di + 1) * M * KJ]
                        z3 = zrow.rearrange("p (m f) -> p m f", m=M)
                        if zdt == "f8act":
                            # ACT upconverts fp8 -> fp16, then adds on fp16
                            p16 = fpool.tile([P, M * KJ], dt, tag="p16")
                            p163 = p16.rearrange("p (m f) -> p m f", m=M)
                            for m in range(M):
                                nc.scalar.copy(p163[:, m, :], z3[:, m, :])
                            z3 = p163
                        elif zdt == "f8mix":
                            # DVE/GPS mixed-dtype add reads fp8 directly
                            p16 = fpool.tile([P, M * KJ], dt, tag="p16")
                            p163 = p16.rearrange("p (m f) -> p m f", m=M)
                            for m in range(M):
                                eng = nc.gpsimd if m < gps_add else nc.vector
                                eng.tensor_add(p163[:, m, :], z3[:, m, :], st)
                            z3 = p163
                        elif zdt == "f8actmix":
                            # GPS: mixed fp8 adds; DVE m's: ACT upconvert
                            # fp8->fp16 then fp16 2x adds on DVE
                            p16 = fpool.tile([P, M * KJ], dt, tag="p16")
                            p163 = p16.rearrange("p (m f) -> p m f", m=M)
                            for m in range(gps_add):
                                nc.gpsimd.tensor_add(
                                    p163[:, m, :], z3[:, m, :], st
                                )
                            for m in range(gps_add, M):
                                nc.scalar.copy(p163[:, m, :], z3[:, m, :])
                            for m in range(gps_add, M):
                                nc.vector.tensor_add(
                                    p163[:, m, :], p163[:, m, :], st
                                )
                            z3 = p163
                        if zdt != "f8mix":
                            # adds: pert = Z + s, in place, per m
                            for m in range(M):
                                eng = nc.gpsimd if m < gps_add else nc.vector
                                eng.tensor_add(z3[:, m, :], z3[:, m, :], st)
                        if fold_depth == 0:
                            for m in range(M):
                                nc.vector.max(
                                    cand[:P, i * 64 + m * 8 : i * 64 + m * 8 + 8],
                                    z3[:, m, :],
                                )
                            continue
                        # fold 1: 800 -> 400 per m.  DVE-added m's first:
                        # GPSIMD adds are slower, so their folds go last to
                        # avoid head-of-line blocking on the DVE queue.
                        ms_order = list(range(gps_add, M)) + list(range(gps_add))
                        f1 = fpool.tile([P, M * (KJ // 2)], dt, tag="f1")
                        f13 = f1.rearrange("p (m f) -> p m f", m=M)
                        for m in ms_order:
                            eng = nc.gpsimd if m < gps_fold1 else nc.vector
                            eng.tensor_max(
                                f13[:, m, :],
                                z3[:, m, : KJ // 2],
                                z3[:, m, KJ // 2 :],
                            )
                        src3 = f13
                        half = KJ // 2
                        if fold_depth >= 2:
                            f2 = fpool.tile([P, M * (KJ // 4)], dt, tag="f2")
                            f23 = f2.rearrange("p (m f) -> p m f", m=M)
                            for m in ms_order:
                                eng = nc.gpsimd if m < gps_fold2 else nc.vector
                                eng.tensor_max(
                                    f23[:, m, :],
                                    src3[:, m, : half // 2],
                                    src3[:, m, half // 2 :],
                                )
                            src3 = f23
                            half = KJ // 4
                        if fold_depth >= 3:
                            f3 = fpool.tile([P, M * (KJ // 8)], dt, tag="f3")
                            f33 = f3.rearrange("p (m f) -> p m f", m=M)
                            for m in range(M):
                                eng = nc.gpsimd if m < gps_fold2 else nc.vector
                                eng.tensor_max(
                                    f33[:, m, :],
                                    src3[:, m, : half // 2],
                                    src3[:, m, half // 2 :],
                                )
                            src3 = f33
                        for m in ms_order:
                            nc.vector.max(
                                cand[:P, i * 64 + m * 8 : i * 64 + m * 8 + 8],
                                src3[:, m, :],
                            )

                if mode != "full":
                    ph2max = wpool.tile([NI * M, 8], dt, tag="ph2max")
                    nc.gpsimd.memset(ph2max[:], 0.0)
                    nc.sync.dma_start(
                        out.ap().flatten().rearrange("(q x) -> q x", x=1),
                        ph2max[:, 4:5],
                    )
                    continue

                stage = dpool.tile([NI, M * 8, 8], dt, tag="stage")
                for i in range(NI):
                    candT = ppool.tile([64, 128], dt, tag="candT")
                    nc.tensor.transpose(
                        candT[:], cand[:, i * 64 : (i + 1) * 64], identsb[:]
                    )
                    nc.vector.max(out8[:, i * 8 : (i + 1) * 8], candT[:])
                    nc.sync.dma_start(stage[:][i], out8[:, i * 8 : (i + 1) * 8])

                ph2 = wpool.tile([NI * M, 64], dt, tag="ph2")
                nc.sync.dma_start(
                    ph2[:], stage[:].flatten().rearrange("(q x) -> q x", q=NI * M)
                )
                ph2max = wpool.tile([NI * M, 8], dt, tag="ph2max")
                nc.vector.max(ph2max[:], ph2[:])
                nc.sync.dma_start(
                    out.ap().flatten().rearrange("(q x) -> q x", x=1),
                    ph2max[:, 4:5],
                )
    return _split_waits(nc) if split else nc


def _make_runner(nc, n_cores):
    import jax
    from jax.experimental.shard_map import shard_map
    from jax.sharding import Mesh, PartitionSpec

    import concourse.mybir as mybir
    from concourse.bass2jax import (
        _bass_exec_p,
        install_neuronx_cc_hook,
        partition_id_tensor,
    )

    install_neuronx_cc_hook()
    partition_name = nc.partition_id_tensor.name if nc.partition_id_tensor else None
    in_names, out_names, out_avals = [], [], []
    for alloc in nc.m.functions[0].allocations:
        if not isinstance(alloc, mybir.MemoryLocationSet):
            continue
        name = alloc.memorylocations[0].name
        if alloc.kind == "ExternalInput":
            if name != partition_name:
                in_names.append(name)
        elif alloc.kind == "ExternalOutput":
            out_names.append(name)
            out_avals.append(
                jax.core.ShapedArray(
                    tuple(alloc.tensor_shape), mybir.dt.np(alloc.dtype)
                )
            )
    n_params = len(in_names)
    all_in = list(in_names) + out_names + ([partition_name] if partition_name else [])

    def _body(*args):
        operands = list(args)
        if partition_name is not None:
            operands.append(partition_id_tensor())
        return tuple(
            _bass_exec_p.bind(
                *operands,
                out_avals=tuple(out_avals),
                in_names=tuple(all_in),
                out_names=tuple(out_names),
                lowering_input_output_aliases=(),
                sim_require_finite=True,
                sim_require_nnan=True,
                nc=nc,
            )
        )

    devices = jax.devices()[:n_cores]
    mesh = Mesh(np.asarray(devices), ("core",))
    n_outs = len(out_names)
    fn = jax.jit(
        shard_map(
            _body,
            mesh=mesh,
            in_specs=(PartitionSpec("core"),) * (n_params + n_outs),
            out_specs=(PartitionSpec("core"),) * n_outs,
            check_rep=False,
        ),
        donate_argnums=tuple(range(n_params, n_params + n_outs)),
        keep_unused=True,
    )
    return fn, in_names, out_names, out_avals


def _get_runner():
    if "runner" not in _CACHE:
        _CACHE["runner"] = _make_runner(_build_nc(), NCORES)
    return _CACHE["runner"]


def _prep_z(Z, zdt=None):
    """[n, d, m] f32/f16 -> per-core [P, NI, M, KJ] contiguous, stacked
    along axis 0 as [NCORES*P, NI, M, KJ] for the sharded runner."""
    import ml_dtypes

    zdt = ZDT if zdt is None else zdt
    npdt = np.float16 if zdt == "f16" else ml_dtypes.float8_e4m3
    Zr = np.asarray(Z).reshape(NCORES, NI, P, KJ, M)
    Zt = Zr.transpose(0, 2, 1, 4, 3)        # [c, P, NI, M, KJ]
    return np.ascontiguousarray(Zt.astype(npdt)).reshape(
        NCORES * P, NI, M, KJ
    )




def _prep_s8(s_f32, y):
    """mask + fp8 + per-core [P, NI*KJ] p-outer stack -> [NCORES*P, NI*KJ]."""
    import ml_dtypes

    rows = np.arange(N)
    s8 = s_f32.astype(ml_dtypes.float8_e4m3)
    s8[rows, np.asarray(y)] = -240.0
    sr = s8.reshape(NCORES, NI, P, KJ).transpose(0, 2, 1, 3)
    return np.ascontiguousarray(sr).reshape(NCORES * P, NI * KJ)


def kernel(s: np.ndarray, y: np.ndarray, Z: np.ndarray) -> np.ndarray:
    s = np.ascontiguousarray(s, dtype=np.float32)
    y = np.asarray(y)
    rows = np.arange(N)
    s_y = s[rows, y]
    if SDT == "f16":
        s_in = s.astype(np.float16)
        s_in[rows, y] = NEG16
    else:
        s_in = _prep_s8(s, y)
    zl = _prep_z(Z)

    arrays = {
        "s": s_in,
        "z": zl,
        "ident": np.tile(np.eye(128, dtype=np.float16), (NCORES, 1)),
    }
    fn, in_names, out_names, out_avals = _get_runner()
    args = [arrays[n] for n in in_names]
    zeros = [
        np.zeros((NCORES * av.shape[0], *av.shape[1:]), av.dtype)
        for av in out_avals
    ]
    outs = fn(*args, *zeros)
    kth = np.asarray(outs[out_names.index("out")], dtype=np.float32)  # (N, M)
    kth_smooth = kth.mean(axis=1, dtype=np.float64)
    loss = np.maximum(1.0 + kth_smooth - s_y.astype(np.float64), 0.0)
    return np.float32(loss.mean())


def measure_hw_time(s, y, Z, reps_list=(16, 256), iters=12, build_kwargs=None):
    import time

    import jax

    build_kwargs = build_kwargs or {}
    s = np.ascontiguousarray(s, dtype=np.float32)
    rows = np.arange(N)
    sdt = build_kwargs.get("sdt") or SDT
    if sdt == "f16":
        s_masked = s.astype(np.float16)
        s_masked[rows, np.asarray(y)] = NEG16
    else:
        s_masked = _prep_s8(s, np.asarray(y))
    zl = _prep_z(Z, zdt=build_kwargs.get("zdt"))
    ident = np.eye(128, dtype=np.float16)
    in_maps = [
        {
            "s": (s_masked[c * NI : (c + 1) * NI] if sdt == "f16"
                  else s_masked[c * P : (c + 1) * P]),
            "z": zl[c * P : (c + 1) * P],
            "ident": ident,
        }
        for c in range(NCORES)
    ]
    results = {}
    for reps in reps_list:
        nc = _build_nc(loop_reps=reps, **build_kwargs)
        fn, in_names, out_names, out_avals = _make_runner(nc, NCORES)
        concat_in = [
            np.concatenate([np.asarray(m[name]) for m in in_maps], axis=0)
            for name in in_names
        ]
        dev_in = [jax.device_put(x) for x in concat_in]
        jax.block_until_ready(dev_in)
        times = []
        for _ in range(iters):
            zeros = [
                jax.device_put(
                    np.zeros((NCORES * av.shape[0], *av.shape[1:]), av.dtype)
                )
                for av in out_avals
            ]
            jax.block_until_ready(zeros)
            t0 = time.perf_counter()
            out = fn(*dev_in, *zeros)
            jax.block_until_ready(out)
            times.append(time.perf_counter() - t0)
        body = sorted(times[1:])
        results[reps] = body[len(body) // 2]
    ks = sorted(results)
    est_ns = None
    if len(ks) >= 2:
        est_ns = (results[ks[-1]] - results[ks[0]]) / (ks[-1] - ks[0]) * 1e9
    return est_ns, results


# revision 3
# speedup vs baseline: 1.4398x; 1.4398x over previous
"""Trainium2 Bass kernel for nn_BalNoisedTopK (balanced noised top-k loss).

loss_i = relu(1 + E_Z[5th-max(s_i^{\\y_i} + Z)] - s_{i,y_i}); out = mean_i.
Pure data parallel over batch: 8 rows/core on 8 cores; host does masking,
dtype/layout prep and the final hinge+means; device does adds + 5th-max.

Measured-driven design (this axon-tunneled TRN2 setup):
  - DMA here costs ~78ns/descriptor + bytes/193GB/s, so both Z and s ship
    as fp8(e4m3) in partition-outermost layouts ([P, NI, M, KJ] m-outer)
    where each partition's slice per DMA is one contiguous run (125
    descriptors/DMA).  fp8 end-to-end rel-err 2.1e-3 vs the 2e-2 budget.
  - pert = Z + s built mostly on the TENSOR engine: psum = I8*z8 + I8*s8
    (two accumulated identity matmuls per PSUM-bank-aligned column block;
    crossing a 2KB PSUM bank corrupts accumulation, hence 512/288 splits
    and 4KB-padded psum pool tiles).  ACT downcasts PSUM fp32 -> fp16.
    GPS_ADD m-samples instead use GPSIMD mixed fp8+fp16 adds, keeping all
    engines busy; the DVE never touches the adds.
  - DVE: two tournament folds (tensor_max of halves, fp16 2x-mode)
    compress 800 -> 200 per partition per (i,m), then the 1x Max8 gives
    the top-8 per chunk.  A global top-5 element of rank j sits at local
    rank <= j of its folded chunk, so the 5th max survives exactly (fold
    collisions ~1e-4 per (i,m); measured zero extra error on real data).
  - Stage-2: PE transpose + cross-partition Max8 + DRAM restage + Max8 =
    exact 5th max of the 1000 surviving candidates per (i,m).
"""

import os
import sys

import numpy as np

for _p in ("/opt/trn_rl_repo", os.path.expanduser("~/.axon_site/_ro/trn_rl_repo")):
    if os.path.isdir(_p) and _p not in sys.path:
        sys.path.insert(0, _p)

N, D, M, K = 64, 100000, 8, 5
NCORES = 8
NI = N // NCORES          # 8 batch rows per core
P = 125                   # SBUF partitions carrying d-chunks
KJ = D // P               # 800 f's per partition
NEG16 = -60000.0
_CACHE = {}

# tuning knobs
FOLD_DEPTH = 2            # folds before Max8 (800 -> 800/2^depth)
GPS_ADD = 2               # how many m's GPSIMD handles for the add
GPS_FOLD1 = 0             # Pool TT-max unsupported by this walrus build
GPS_FOLD2 = 0
ZROWS = 2                 # batch rows per Z DMA
ZBUFS = 4                 # z tile pool buffers (in units of ZROWS rows)
ZDT = "f8pe"                # f16 | f8 (SWDGE cast) | f8act (ACT upconvert)
                          # | f8mix (DVE mixed-dtype adds read fp8 directly)
ZDMA = "gpsimd"           # engine for Z DMA: sync (HWDGE) or gpsimd (SWDGE)
SDT = "f8"               # s HBM dtype: f16, or f8 (ACT upconverts once/iter)


def _split_waits(nc, max_waits=1):
    import concourse.mybir as mybir

    for blk in nc.m.functions[0].blocks:
        new_list = []
        for inst in blk.instructions:
            si = inst.sync_info
            if si is not None and len(si.on_wait) > max_waits:
                waits = list(si.on_wait)
                keep = [w for w in waits if w.wait_reg is not None]
                movable = [w for w in waits if w.wait_reg is None]
                while len(keep) < max_waits and movable:
                    keep.append(movable.pop())
                k = 0
                while movable:
                    chunk, movable = movable[:max_waits], movable[max_waits:]
                    ev = mybir.InstEventSemaphore(
                        name=f"{inst.name}_xw{k}", ins=[], outs=[]
                    )
                    ev.engine = inst.engine
                    ev.sync_info = mybir.SyncInfo(on_wait=chunk, on_update=[])
                    new_list.append(ev)
                    k += 1
                inst.sync_info = mybir.SyncInfo(
                    on_wait=keep, on_update=list(si.on_update)
                )
            new_list.append(inst)
        blk.instructions = new_list
    return nc


def _build_nc(reps=1, split=True, mode="full", loop_reps=0,
              fold_depth=None, gps_add=None, gps_fold1=None, gps_fold2=None,
              zrows=None, zbufs=None, zdt=None, zdma=None, sdt=None):
    import concourse.bass as bass
    import concourse.mybir as mybir
    from concourse.tile import TileContext

    fold_depth = FOLD_DEPTH if fold_depth is None else fold_depth
    gps_add = GPS_ADD if gps_add is None else gps_add
    gps_fold1 = GPS_FOLD1 if gps_fold1 is None else gps_fold1
    gps_fold2 = GPS_FOLD2 if gps_fold2 is None else gps_fold2
    zrows = ZROWS if zrows is None else zrows
    zbufs = ZBUFS if zbufs is None else zbufs
    zdt = ZDT if zdt is None else zdt
    zdma = ZDMA if zdma is None else zdma
    sdt = SDT if sdt is None else sdt

    dt = mybir.dt.float16
    zdram_dt = mybir.dt.float16 if zdt == "f16" else mybir.dt.float8e4
    zsb_dt = dt if zdt in ("f16", "f8") else mybir.dt.float8e4
    nc = bass.Bass("TRN2")
    # z layout: host-prepped [P, NI, M, KJ] (partition-outermost:
    # each partition's (i, m, f) range is one contiguous DRAM run)
    if sdt == "f16":
        s = nc.dram_tensor("s", (NI, D), dt, kind="ExternalInput")
    else:
        s = nc.dram_tensor("s", (P, NI * KJ), mybir.dt.float8e4,
                           kind="ExternalInput")
    z = nc.dram_tensor("z", (P, NI, M, KJ), zdram_dt, kind="ExternalInput")
    ident = nc.dram_tensor("ident", (128, 128), dt, kind="ExternalInput")
    ident8 = nc.dram_tensor("ident8", (128, 128), mybir.dt.float8e4,
                            kind="ExternalInput")
    out = nc.dram_tensor("out", (NI, M), dt, kind="ExternalOutput")

    fsz = KJ >> fold_depth      # Max8 input size per (m, partition)

    with TileContext(nc) as tc:
        with (
            tc.tile_pool(name="zpool", bufs=zbufs) as zpool,
            tc.tile_pool(name="fpool", bufs=4) as fpool,
            tc.tile_pool(name="cpool", bufs=1) as cpool,
            tc.tile_pool(name="wpool", bufs=2) as wpool,
            tc.tile_pool(name="ppool", bufs=2, space="PSUM") as ppool,
            tc.tile_pool(name="apool", bufs=3, space="PSUM") as apool,
            tc.tile_pool(name="dpool", bufs=1, space="DRAM") as dpool,
        ):
            identsb = cpool.tile([128, 128], dt)
            nc.sync.dma_start(identsb[:], ident.ap())
            ident8sb = cpool.tile([128, 128], mybir.dt.float8e4)
            nc.sync.dma_start(ident8sb[:], ident8.ap())

            import contextlib

            loop_cm = (
                tc.For_i(0, loop_reps, 1)
                if loop_reps > 0
                else contextlib.nullcontext()
            )
            with loop_cm:
              for _rep in range(reps):
                # all rows' s in one DMA: st_all[p, i*KJ+f] <- s[i, p*KJ+f]
                st_all = wpool.tile([P, NI * KJ], dt, tag="st_all")
                if sdt == "f16":
                    s_src = s.ap().rearrange("i (p f) -> p i f", p=P)
                    nc.sync.dma_start(
                        st_all[:].rearrange("p (i f) -> p i f", i=NI), s_src
                    )
                else:
                    st8_all = wpool.tile([P, NI * KJ], mybir.dt.float8e4,
                                         tag="st8")
                    nc.sync.dma_start(st8_all[:], s.ap())
                    nc.scalar.copy(st_all[:], st8_all[:])
                cand = wpool.tile([128, NI * M * 8], dt, tag="cand")
                nc.gpsimd.memset(cand[:], NEG16)
                out8 = wpool.tile([64, NI * 8], dt, tag="out8")

                for i0 in range(0, NI, zrows):
                    zt = zpool.tile([P, zrows * M * KJ], zsb_dt, tag="zt")
                    if mode == "compute":
                        nc.gpsimd.memset(zt[:], 0.0)
                    if mode != "compute":
                        # one DMA for zrows rows; contiguous per partition
                        zsrc = z.ap()[:, i0 : i0 + zrows]
                        zdst = zt[:].rearrange(
                            "p (i m f) -> p i m f", i=zrows, m=M
                        )
                        if zdt == "f8" or (zdt == "f16" and zdma == "gpsimd"):
                            # SWDGE path (casts fp8 -> fp16 inline for f8)
                            nc.gpsimd.dma_start(zdst, zsrc)
                        else:
                            nc.sync.dma_start(zdst, zsrc)
                    if mode == "dma":
                        continue
                    for di in range(zrows):
                        i = i0 + di
                        st = st_all[:, i * KJ : (i + 1) * KJ]
                        zrow = zt[:, di * M * KJ : (di + 1) * M * KJ]
                        z3 = zrow.rearrange("p (m f) -> p m f", m=M)
                        if zdt == "f8act":
                            # ACT upconverts fp8 -> fp16, then adds on fp16
                            p16 = fpool.tile([P, M * KJ], dt, tag="p16")
                            p163 = p16.rearrange("p (m f) -> p m f", m=M)
                            for m in range(M):
                                nc.scalar.copy(p163[:, m, :], z3[:, m, :])
                            z3 = p163
                        elif zdt == "f8mix":
                            # DVE/GPS mixed-dtype add reads fp8 directly
                            p16 = fpool.tile([P, M * KJ], dt, tag="p16")
                            p163 = p16.rearrange("p (m f) -> p m f", m=M)
                            for m in range(M):
                                eng = nc.gpsimd if m < gps_add else nc.vector
                                eng.tensor_add(p163[:, m, :], z3[:, m, :], st)
                            z3 = p163
                        elif zdt == "f8pe":
                            # PE: psum = I8*z8 + I8*s8 (bank-aligned column
                            # blocks), ACT downcasts PSUM fp32 -> fp16.
                            # GPS m's: mixed fp8-z + fp16-s adds.
                            p16 = fpool.tile([P, M * KJ], dt, tag="p16")
                            p163 = p16.rearrange("p (m f) -> p m f", m=M)
                            for m in range(gps_add):
                                nc.gpsimd.tensor_add(
                                    p163[:, m, :], z3[:, m, :], st
                                )
                            st8r = st8_all[:, i * KJ : (i + 1) * KJ]
                            for m in range(gps_add, M):
                                # 1024 f32 = 4KB = exactly 2 PSUM banks ->
                                # every pool buffer stays bank-aligned
                                psb = apool.tile([P, 1024], mybir.dt.float32,
                                                 tag="ps")
                                ps = psb[:, :KJ]
                                for c0, c1 in ((0, 512), (512, KJ)):
                                    nc.tensor.matmul(
                                        ps[:, c0:c1], ident8sb[:P, :P],
                                        z3[:, m, c0:c1],
                                        start=True, stop=False,
                                    )
                                    nc.tensor.matmul(
                                        ps[:, c0:c1], ident8sb[:P, :P],
                                        st8r[:, c0:c1],
                                        start=False, stop=True,
                                    )
                                nc.scalar.copy(p163[:, m, :], ps[:])
                            z3 = p163
                        elif zdt == "f8actmix":
                            # GPS: mixed fp8 adds; DVE m's: ACT upconvert
                            # fp8->fp16 then fp16 2x adds on DVE
                            p16 = fpool.tile([P, M * KJ], dt, tag="p16")
                            p163 = p16.rearrange("p (m f) -> p m f", m=M)
                            for m in range(gps_add):
                                nc.gpsimd.tensor_add(
                                    p163[:, m, :], z3[:, m, :], st
                                )
                            for m in range(gps_add, M):
                                nc.scalar.copy(p163[:, m, :], z3[:, m, :])
                            for m in range(gps_add, M):
                                nc.vector.tensor_add(
                                    p163[:, m, :], p163[:, m, :], st
                                )
                            z3 = p163
                        if zdt in ("f16", "f8", "f8act"):
                            # adds: pert = Z + s, in place, per m
                            for m in range(M):
                                eng = nc.gpsimd if m < gps_add else nc.vector
                                eng.tensor_add(z3[:, m, :], z3[:, m, :], st)
                        if fold_depth == 0:
                            for m in range(M):
                                nc.vector.max(
                                    cand[:P, i * 64 + m * 8 : i * 64 + m * 8 + 8],
                                    z3[:, m, :],
                                )
                            continue
                        # fold 1: 800 -> 400 per m.  DVE-added m's first:
                        # GPSIMD adds are slower, so their folds go last to
                        # avoid head-of-line blocking on the DVE queue.
                        ms_order = list(range(gps_add, M)) + list(range(gps_add))
                        f1 = fpool.tile([P, M * (KJ // 2)], dt, tag="f1")
                        f13 = f1.rearrange("p (m f) -> p m f", m=M)
                        for m in ms_order:
                            eng = nc.gpsimd if m < gps_fold1 else nc.vector
                            eng.tensor_max(
                                f13[:, m, :],
                                z3[:, m, : KJ // 2],
                                z3[:, m, KJ // 2 :],
                            )
                        src3 = f13
                        half = KJ // 2
                        if fold_depth >= 2:
                            f2 = fpool.tile([P, M * (KJ // 4)], dt, tag="f2")
                            f23 = f2.rearrange("p (m f) -> p m f", m=M)
                            for m in ms_order:
                                eng = nc.gpsimd if m < gps_fold2 else nc.vector
                                eng.tensor_max(
                                    f23[:, m, :],
                                    src3[:, m, : half // 2],
                                    src3[:, m, half // 2 :],
                                )
                            src3 = f23
                            half = KJ // 4
                        if fold_depth >= 3:
                            f3 = fpool.tile([P, M * (KJ // 8)], dt, tag="f3")
                            f33 = f3.rearrange("p (m f) -> p m f", m=M)
                            for m in range(M):
                                eng = nc.gpsimd if m < gps_fold2 else nc.vector
                                eng.tensor_max(
                                    f33[:, m, :],
                                    src3[:, m, : half // 2],
                                    src3[:, m, half // 2 :],
                                )
                            src3 = f33
                        for m in ms_order:
                            nc.vector.max(
                                cand[:P, i * 64 + m * 8 : i * 64 + m * 8 + 8],
                                src3[:, m, :],
                            )

                if mode != "full":
                    ph2max = wpool.tile([NI * M, 8], dt, tag="ph2max")
                    nc.gpsimd.memset(ph2max[:], 0.0)
                    nc.sync.dma_start(
                        out.ap().flatten().rearrange("(q x) -> q x", x=1),
                        ph2max[:, 4:5],
                    )
                    continue

                stage = dpool.tile([NI, M * 8, 8], dt, tag="stage")
                for i in range(NI):
                    candT = ppool.tile([64, 128], dt, tag="candT")
                    nc.tensor.transpose(
                        candT[:], cand[:, i * 64 : (i + 1) * 64], identsb[:]
                    )
                    nc.vector.max(out8[:, i * 8 : (i + 1) * 8], candT[:])
                    nc.sync.dma_start(stage[:][i], out8[:, i * 8 : (i + 1) * 8])

                ph2 = wpool.tile([NI * M, 64], dt, tag="ph2")
                nc.sync.dma_start(
                    ph2[:], stage[:].flatten().rearrange("(q x) -> q x", q=NI * M)
                )
                ph2max = wpool.tile([NI * M, 8], dt, tag="ph2max")
                nc.vector.max(ph2max[:], ph2[:])
                nc.sync.dma_start(
                    out.ap().flatten().rearrange("(q x) -> q x", x=1),
                    ph2max[:, 4:5],
                )
    return _split_waits(nc) if split else nc


def _make_runner(nc, n_cores):
    import jax
    from jax.experimental.shard_map import shard_map
    from jax.sharding import Mesh, PartitionSpec

    import concourse.mybir as mybir
    from concourse.bass2jax import (
        _bass_exec_p,
        install_neuronx_cc_hook,
        partition_id_tensor,
    )

    install_neuronx_cc_hook()
    partition_name = nc.partition_id_tensor.name if nc.partition_id_tensor else None
    in_names, out_names, out_avals = [], [], []
    for alloc in nc.m.functions[0].allocations:
        if not isinstance(alloc, mybir.MemoryLocationSet):
            continue
        name = alloc.memorylocations[0].name
        if alloc.kind == "ExternalInput":
            if name != partition_name:
                in_names.append(name)
        elif alloc.kind == "ExternalOutput":
            out_names.append(name)
            out_avals.append(
                jax.core.ShapedArray(
                    tuple(alloc.tensor_shape), mybir.dt.np(alloc.dtype)
                )
            )
    n_params = len(in_names)
    all_in = list(in_names) + out_names + ([partition_name] if partition_name else [])

    def _body(*args):
        operands = list(args)
        if partition_name is not None:
            operands.append(partition_id_tensor())
        return tuple(
            _bass_exec_p.bind(
                *operands,
                out_avals=tuple(out_avals),
                in_names=tuple(all_in),
                out_names=tuple(out_names),
                lowering_input_output_aliases=(),
                sim_require_finite=True,
                sim_require_nnan=True,
                nc=nc,
            )
        )

    devices = jax.devices()[:n_cores]
    mesh = Mesh(np.asarray(devices), ("core",))
    n_outs = len(out_names)
    fn = jax.jit(
        shard_map(
            _body,
            mesh=mesh,
            in_specs=(PartitionSpec("core"),) * (n_params + n_outs),
            out_specs=(PartitionSpec("core"),) * n_outs,
            check_rep=False,
        ),
        donate_argnums=tuple(range(n_params, n_params + n_outs)),
        keep_unused=True,
    )
    return fn, in_names, out_names, out_avals


def _get_runner():
    if "runner" not in _CACHE:
        _CACHE["runner"] = _make_runner(_build_nc(), NCORES)
    return _CACHE["runner"]


def _prep_z(Z, zdt=None):
    """[n, d, m] f32/f16 -> per-core [P, NI, M, KJ] contiguous, stacked
    along axis 0 as [NCORES*P, NI, M, KJ] for the sharded runner."""
    import ml_dtypes

    zdt = ZDT if zdt is None else zdt
    npdt = np.float16 if zdt == "f16" else ml_dtypes.float8_e4m3
    Zr = np.asarray(Z).reshape(NCORES, NI, P, KJ, M)
    Zt = Zr.transpose(0, 2, 1, 4, 3)        # [c, P, NI, M, KJ]
    return np.ascontiguousarray(Zt.astype(npdt)).reshape(
        NCORES * P, NI, M, KJ
    )




def _prep_s8(s_f32, y):
    """mask + fp8 + per-core [P, NI*KJ] p-outer stack -> [NCORES*P, NI*KJ]."""
    import ml_dtypes

    rows = np.arange(N)
    s8 = s_f32.astype(ml_dtypes.float8_e4m3)
    s8[rows, np.asarray(y)] = -240.0
    sr = s8.reshape(NCORES, NI, P, KJ).transpose(0, 2, 1, 3)
    return np.ascontiguousarray(sr).reshape(NCORES * P, NI * KJ)


def kernel(s: np.ndarray, y: np.ndarray, Z: np.ndarray) -> np.ndarray:
    s = np.ascontiguousarray(s, dtype=np.float32)
    y = np.asarray(y)
    rows = np.arange(N)
    s_y = s[rows, y]
    if SDT == "f16":
        s_in = s.astype(np.float16)
        s_in[rows, y] = NEG16
    else:
        s_in = _prep_s8(s, y)
    zl = _prep_z(Z)

    import ml_dtypes as _mld

    arrays = {
        "s": s_in,
        "z": zl,
        "ident": np.tile(np.eye(128, dtype=np.float16), (NCORES, 1)),
        "ident8": np.tile(
            np.eye(128).astype(_mld.float8_e4m3), (NCORES, 1)
        ),
    }
    fn, in_names, out_names, out_avals = _get_runner()
    args = [arrays[n] for n in in_names]
    zeros = [
        np.zeros((NCORES * av.shape[0], *av.shape[1:]), av.dtype)
        for av in out_avals
    ]
    outs = fn(*args, *zeros)
    kth = np.asarray(outs[out_names.index("out")], dtype=np.float32)  # (N, M)
    kth_smooth = kth.mean(axis=1, dtype=np.float64)
    loss = np.maximum(1.0 + kth_smooth - s_y.astype(np.float64), 0.0)
    return np.float32(loss.mean())


def measure_hw_time(s, y, Z, reps_list=(16, 256), iters=12, build_kwargs=None):
    import time

    import jax

    build_kwargs = build_kwargs or {}
    s = np.ascontiguousarray(s, dtype=np.float32)
    rows = np.arange(N)
    sdt = build_kwargs.get("sdt") or SDT
    if sdt == "f16":
        s_masked = s.astype(np.float16)
        s_masked[rows, np.asarray(y)] = NEG16
    else:
        s_masked = _prep_s8(s, np.asarray(y))
    zl = _prep_z(Z, zdt=build_kwargs.get("zdt"))
    ident = np.eye(128, dtype=np.float16)
    import ml_dtypes as _mld

    ident8 = np.eye(128).astype(_mld.float8_e4m3)
    in_maps = [
        {
            "s": (s_masked[c * NI : (c + 1) * NI] if sdt == "f16"
                  else s_masked[c * P : (c + 1) * P]),
            "z": zl[c * P : (c + 1) * P],
            "ident": ident,
            "ident8": ident8,
        }
        for c in range(NCORES)
    ]
    results = {}
    for reps in reps_list:
        nc = _build_nc(loop_reps=reps, **build_kwargs)
        fn, in_names, out_names, out_avals = _make_runner(nc, NCORES)
        concat_in = [
            np.concatenate([np.asarray(m[name]) for m in in_maps], axis=0)
            for name in in_names
        ]
        dev_in = [jax.device_put(x) for x in concat_in]
        jax.block_until_ready(dev_in)
        times = []
        for _ in range(iters):
            zeros = [
                jax.device_put(
                    np.zeros((NCORES * av.shape[0], *av.shape[1:]), av.dtype)
                )
                for av in out_avals
            ]
            jax.block_until_ready(zeros)
            t0 = time.perf_counter()
            out = fn(*dev_in, *zeros)
            jax.block_until_ready(out)
            times.append(time.perf_counter() - t0)
        body = sorted(times[1:])
        results[reps] = body[len(body) // 2]
    ks = sorted(results)
    est_ns = None
    if len(ks) >= 2:
        est_ns = (results[ks[-1]] - results[ks[0]]) / (ks[-1] - ks[0]) * 1e9
    return est_ns, results


# revision 4
# speedup vs baseline: 1.8224x; 1.2657x over previous
"""Trainium2 Bass kernel for nn_BalNoisedTopK (balanced noised top-k loss).

loss_i = relu(1 + E_Z[5th-max(s_i^{\\y_i} + Z)] - s_{i,y_i}); out = mean_i.
Pure data parallel over batch: 8 rows/core on 8 cores; host does masking,
dtype/layout prep and the final hinge+means; device does adds + 5th-max.

Measured-driven design (this axon-tunneled TRN2 setup):
  - DMA here costs ~78ns/descriptor + bytes/193GB/s, so both Z and s ship
    as fp8(e4m3) in partition-outermost layouts ([P, NI, M, KJ] m-outer)
    where each partition's slice per DMA is one contiguous run (125
    descriptors/DMA).  fp8 end-to-end rel-err 2.1e-3 vs the 2e-2 budget.
  - pert = Z + s built mostly on the TENSOR engine: psum = I8*z8 + I8*s8
    (two accumulated identity matmuls per PSUM-bank-aligned column block;
    crossing a 2KB PSUM bank corrupts accumulation, hence 512/288 splits
    and 4KB-padded psum pool tiles).  ACT downcasts PSUM fp32 -> fp16.
    GPS_ADD m-samples instead use GPSIMD mixed fp8+fp16 adds, keeping all
    engines busy; the DVE never touches the adds.
  - DVE: two tournament folds (tensor_max of halves, fp16 2x-mode)
    compress 800 -> 200 per partition per (i,m), then the 1x Max8 gives
    the top-8 per chunk.  A global top-5 element of rank j sits at local
    rank <= j of its folded chunk, so the 5th max survives exactly (fold
    collisions ~1e-4 per (i,m); measured zero extra error on real data).
  - Stage-2: PE transpose + cross-partition Max8 + DRAM restage + Max8 =
    exact 5th max of the 1000 surviving candidates per (i,m).
"""

import os
import sys

import numpy as np

for _p in ("/opt/trn_rl_repo", os.path.expanduser("~/.axon_site/_ro/trn_rl_repo")):
    if os.path.isdir(_p) and _p not in sys.path:
        sys.path.insert(0, _p)

N, D, M, K = 64, 100000, 8, 5
NCORES = 8
NI = N // NCORES          # 8 batch rows per core
P = 125                   # SBUF partitions carrying d-chunks
KJ = D // P               # 800 f's per partition
NEG16 = -60000.0
_CACHE = {}

# tuning knobs
FOLD_DEPTH = 2            # folds before Max8 (800 -> 800/2^depth)
GPS_ADD = 0               # how many m's GPSIMD handles for the add
GPS_FOLD1 = 0             # Pool TT-max unsupported by this walrus build
GPS_FOLD2 = 0
ZROWS = 2                 # batch rows per Z DMA
ZBUFS = 4                 # z tile pool buffers (in units of ZROWS rows)
ZDT = "f8pe"                # f16 | f8 (SWDGE cast) | f8act (ACT upconvert)
                          # | f8mix (DVE mixed-dtype adds read fp8 directly)
ZDMA = "gpsimd"           # engine for Z DMA: sync (HWDGE) or gpsimd (SWDGE)
SDT = "f8"               # s HBM dtype: f16, or f8 (ACT upconverts once/iter)


def _split_waits(nc, max_waits=1):
    import concourse.mybir as mybir

    for blk in nc.m.functions[0].blocks:
        new_list = []
        for inst in blk.instructions:
            si = inst.sync_info
            if si is not None and len(si.on_wait) > max_waits:
                waits = list(si.on_wait)
                keep = [w for w in waits if w.wait_reg is not None]
                movable = [w for w in waits if w.wait_reg is None]
                while len(keep) < max_waits and movable:
                    keep.append(movable.pop())
                k = 0
                while movable:
                    chunk, movable = movable[:max_waits], movable[max_waits:]
                    ev = mybir.InstEventSemaphore(
                        name=f"{inst.name}_xw{k}", ins=[], outs=[]
                    )
                    ev.engine = inst.engine
                    ev.sync_info = mybir.SyncInfo(on_wait=chunk, on_update=[])
                    new_list.append(ev)
                    k += 1
                inst.sync_info = mybir.SyncInfo(
                    on_wait=keep, on_update=list(si.on_update)
                )
            new_list.append(inst)
        blk.instructions = new_list
    return nc


def _build_nc(reps=1, split=True, mode="full", loop_reps=0,
              fold_depth=None, gps_add=None, gps_fold1=None, gps_fold2=None,
              zrows=None, zbufs=None, zdt=None, zdma=None, sdt=None):
    import concourse.bass as bass
    import concourse.mybir as mybir
    from concourse.tile import TileContext

    fold_depth = FOLD_DEPTH if fold_depth is None else fold_depth
    gps_add = GPS_ADD if gps_add is None else gps_add
    gps_fold1 = GPS_FOLD1 if gps_fold1 is None else gps_fold1
    gps_fold2 = GPS_FOLD2 if gps_fold2 is None else gps_fold2
    zrows = ZROWS if zrows is None else zrows
    zbufs = ZBUFS if zbufs is None else zbufs
    zdt = ZDT if zdt is None else zdt
    zdma = ZDMA if zdma is None else zdma
    sdt = SDT if sdt is None else sdt

    dt = mybir.dt.float16
    zdram_dt = mybir.dt.float16 if zdt == "f16" else mybir.dt.float8e4
    zsb_dt = dt if zdt in ("f16", "f8") else mybir.dt.float8e4
    nc = bass.Bass("TRN2")
    # z layout: host-prepped [P, NI, M, KJ] (partition-outermost:
    # each partition's (i, m, f) range is one contiguous DRAM run)
    if sdt == "f16":
        s = nc.dram_tensor("s", (NI, D), dt, kind="ExternalInput")
    else:
        s = nc.dram_tensor("s", (P, NI * KJ), mybir.dt.float8e4,
                           kind="ExternalInput")
    z = nc.dram_tensor("z", (P, NI, M, KJ), zdram_dt, kind="ExternalInput")
    ident = nc.dram_tensor("ident", (128, 128), dt, kind="ExternalInput")
    ident8 = nc.dram_tensor("ident8", (128, 128), mybir.dt.float8e4,
                            kind="ExternalInput")
    out = nc.dram_tensor("out", (NI, M), dt, kind="ExternalOutput")

    fsz = KJ >> fold_depth      # Max8 input size per (m, partition)

    with TileContext(nc) as tc:
        with (
            tc.tile_pool(name="zpool", bufs=zbufs) as zpool,
            tc.tile_pool(name="fpool", bufs=4) as fpool,
            tc.tile_pool(name="cpool", bufs=1) as cpool,
            tc.tile_pool(name="wpool", bufs=2) as wpool,
            tc.tile_pool(name="ppool", bufs=2, space="PSUM") as ppool,
            tc.tile_pool(name="apool", bufs=3, space="PSUM") as apool,
            tc.tile_pool(name="dpool", bufs=1, space="DRAM") as dpool,
        ):
            identsb = cpool.tile([128, 128], dt)
            nc.sync.dma_start(identsb[:], ident.ap())
            ident8sb = cpool.tile([128, 128], mybir.dt.float8e4)
            nc.sync.dma_start(ident8sb[:], ident8.ap())

            import contextlib

            loop_cm = (
                tc.For_i(0, loop_reps, 1)
                if loop_reps > 0
                else contextlib.nullcontext()
            )
            with loop_cm:
              for _rep in range(reps):
                # all rows' s in one DMA: st_all[p, i*KJ+f] <- s[i, p*KJ+f]
                st_all = wpool.tile([P, NI * KJ], dt, tag="st_all")
                if sdt == "f16":
                    s_src = s.ap().rearrange("i (p f) -> p i f", p=P)
                    nc.sync.dma_start(
                        st_all[:].rearrange("p (i f) -> p i f", i=NI), s_src
                    )
                else:
                    st8_all = wpool.tile([P, NI * KJ], mybir.dt.float8e4,
                                         tag="st8")
                    nc.sync.dma_start(st8_all[:], s.ap())
                    nc.scalar.copy(st_all[:], st8_all[:])
                cand = wpool.tile([128, NI * M * 8], dt, tag="cand")
                nc.gpsimd.memset(cand[:], NEG16)
                out8 = wpool.tile([64, NI * 8], dt, tag="out8")

                for i0 in range(0, NI, zrows):
                    zt = zpool.tile([P, zrows * M * KJ], zsb_dt, tag="zt")
                    if mode == "compute":
                        nc.gpsimd.memset(zt[:], 0.0)
                    if mode != "compute":
                        # one DMA for zrows rows; contiguous per partition
                        zsrc = z.ap()[:, i0 : i0 + zrows]
                        zdst = zt[:].rearrange(
                            "p (i m f) -> p i m f", i=zrows, m=M
                        )
                        if zdt == "f8" or (zdt == "f16" and zdma == "gpsimd"):
                            # SWDGE path (casts fp8 -> fp16 inline for f8)
                            nc.gpsimd.dma_start(zdst, zsrc)
                        else:
                            nc.sync.dma_start(zdst, zsrc)
                    if mode == "dma":
                        continue
                    for di in range(zrows):
                        i = i0 + di
                        st = st_all[:, i * KJ : (i + 1) * KJ]
                        zrow = zt[:, di * M * KJ : (di + 1) * M * KJ]
                        z3 = zrow.rearrange("p (m f) -> p m f", m=M)
                        if zdt == "f8act":
                            # ACT upconverts fp8 -> fp16, then adds on fp16
                            p16 = fpool.tile([P, M * KJ], dt, tag="p16")
                            p163 = p16.rearrange("p (m f) -> p m f", m=M)
                            for m in range(M):
                                nc.scalar.copy(p163[:, m, :], z3[:, m, :])
                            z3 = p163
                        elif zdt == "f8mix":
                            # DVE/GPS mixed-dtype add reads fp8 directly
                            p16 = fpool.tile([P, M * KJ], dt, tag="p16")
                            p163 = p16.rearrange("p (m f) -> p m f", m=M)
                            for m in range(M):
                                eng = nc.gpsimd if m < gps_add else nc.vector
                                eng.tensor_add(p163[:, m, :], z3[:, m, :], st)
                            z3 = p163
                        elif zdt == "f8pe":
                            # PE: psum = I8*z8 + I8*s8 (bank-aligned column
                            # blocks), ACT downcasts PSUM fp32 -> fp16.
                            # GPS m's: mixed fp8-z + fp16-s adds.
                            p16 = fpool.tile([P, M * KJ], dt, tag="p16")
                            p163 = p16.rearrange("p (m f) -> p m f", m=M)
                            for m in range(gps_add):
                                nc.gpsimd.tensor_add(
                                    p163[:, m, :], z3[:, m, :], st
                                )
                            st8r = st8_all[:, i * KJ : (i + 1) * KJ]
                            for m in range(gps_add, M):
                                # 1024 f32 = 4KB = exactly 2 PSUM banks ->
                                # every pool buffer stays bank-aligned
                                psb = apool.tile([P, 1024], mybir.dt.float32,
                                                 tag="ps")
                                ps = psb[:, :KJ]
                                for c0, c1 in ((0, 512), (512, KJ)):
                                    nc.tensor.matmul(
                                        ps[:, c0:c1], ident8sb[:P, :P],
                                        z3[:, m, c0:c1],
                                        start=True, stop=False,
                                    )
                                    nc.tensor.matmul(
                                        ps[:, c0:c1], ident8sb[:P, :P],
                                        st8r[:, c0:c1],
                                        start=False, stop=True,
                                    )
                                nc.scalar.copy(p163[:, m, :], ps[:])
                            z3 = p163
                        elif zdt == "f8actmix":
                            # GPS: mixed fp8 adds; DVE m's: ACT upconvert
                            # fp8->fp16 then fp16 2x adds on DVE
                            p16 = fpool.tile([P, M * KJ], dt, tag="p16")
                            p163 = p16.rearrange("p (m f) -> p m f", m=M)
                            for m in range(gps_add):
                                nc.gpsimd.tensor_add(
                                    p163[:, m, :], z3[:, m, :], st
                                )
                            for m in range(gps_add, M):
                                nc.scalar.copy(p163[:, m, :], z3[:, m, :])
                            for m in range(gps_add, M):
                                nc.vector.tensor_add(
                                    p163[:, m, :], p163[:, m, :], st
                                )
                            z3 = p163
                        if zdt in ("f16", "f8", "f8act"):
                            # adds: pert = Z + s, in place, per m
                            for m in range(M):
                                eng = nc.gpsimd if m < gps_add else nc.vector
                                eng.tensor_add(z3[:, m, :], z3[:, m, :], st)
                        if fold_depth == 0:
                            for m in range(M):
                                nc.vector.max(
                                    cand[:P, i * 64 + m * 8 : i * 64 + m * 8 + 8],
                                    z3[:, m, :],
                                )
                            continue
                        # fold 1: 800 -> 400 per m.  DVE-added m's first:
                        # GPSIMD adds are slower, so their folds go last to
                        # avoid head-of-line blocking on the DVE queue.
                        ms_order = list(range(gps_add, M)) + list(range(gps_add))
                        f1 = fpool.tile([P, M * (KJ // 2)], dt, tag="f1")
                        f13 = f1.rearrange("p (m f) -> p m f", m=M)
                        for m in ms_order:
                            eng = nc.gpsimd if m < gps_fold1 else nc.vector
                            eng.tensor_max(
                                f13[:, m, :],
                                z3[:, m, : KJ // 2],
                                z3[:, m, KJ // 2 :],
                            )
                        src3 = f13
                        half = KJ // 2
                        if fold_depth >= 2:
                            f2 = fpool.tile([P, M * (KJ // 4)], dt, tag="f2")
                            f23 = f2.rearrange("p (m f) -> p m f", m=M)
                            for m in ms_order:
                                eng = nc.gpsimd if m < gps_fold2 else nc.vector
                                eng.tensor_max(
                                    f23[:, m, :],
                                    src3[:, m, : half // 2],
                                    src3[:, m, half // 2 :],
                                )
                            src3 = f23
                            half = KJ // 4
                        if fold_depth >= 3:
                            f3 = fpool.tile([P, M * (KJ // 8)], dt, tag="f3")
                            f33 = f3.rearrange("p (m f) -> p m f", m=M)
                            for m in range(M):
                                eng = nc.gpsimd if m < gps_fold2 else nc.vector
                                eng.tensor_max(
                                    f33[:, m, :],
                                    src3[:, m, : half // 2],
                                    src3[:, m, half // 2 :],
                                )
                            src3 = f33
                        for m in ms_order:
                            nc.vector.max(
                                cand[:P, i * 64 + m * 8 : i * 64 + m * 8 + 8],
                                src3[:, m, :],
                            )

                if mode != "full":
                    ph2max = wpool.tile([NI * M, 8], dt, tag="ph2max")
                    nc.gpsimd.memset(ph2max[:], 0.0)
                    nc.sync.dma_start(
                        out.ap().flatten().rearrange("(q x) -> q x", x=1),
                        ph2max[:, 4:5],
                    )
                    continue

                stage = dpool.tile([NI, M * 8, 8], dt, tag="stage")
                for i in range(NI):
                    candT = ppool.tile([64, 128], dt, tag="candT")
                    nc.tensor.transpose(
                        candT[:], cand[:, i * 64 : (i + 1) * 64], identsb[:]
                    )
                    nc.vector.max(out8[:, i * 8 : (i + 1) * 8], candT[:])
                    nc.sync.dma_start(stage[:][i], out8[:, i * 8 : (i + 1) * 8])

                ph2 = wpool.tile([NI * M, 64], dt, tag="ph2")
                nc.sync.dma_start(
                    ph2[:], stage[:].flatten().rearrange("(q x) -> q x", q=NI * M)
                )
                ph2max = wpool.tile([NI * M, 8], dt, tag="ph2max")
                nc.vector.max(ph2max[:], ph2[:])
                nc.sync.dma_start(
                    out.ap().flatten().rearrange("(q x) -> q x", x=1),
                    ph2max[:, 4:5],
                )
    return _split_waits(nc) if split else nc


def _make_runner(nc, n_cores):
    import jax
    from jax.experimental.shard_map import shard_map
    from jax.sharding import Mesh, PartitionSpec

    import concourse.mybir as mybir
    from concourse.bass2jax import (
        _bass_exec_p,
        install_neuronx_cc_hook,
        partition_id_tensor,
    )

    install_neuronx_cc_hook()
    partition_name = nc.partition_id_tensor.name if nc.partition_id_tensor else None
    in_names, out_names, out_avals = [], [], []
    for alloc in nc.m.functions[0].allocations:
        if not isinstance(alloc, mybir.MemoryLocationSet):
            continue
        name = alloc.memorylocations[0].name
        if alloc.kind == "ExternalInput":
            if name != partition_name:
                in_names.append(name)
        elif alloc.kind == "ExternalOutput":
            out_names.append(name)
            out_avals.append(
                jax.core.ShapedArray(
                    tuple(alloc.tensor_shape), mybir.dt.np(alloc.dtype)
                )
            )
    n_params = len(in_names)
    all_in = list(in_names) + out_names + ([partition_name] if partition_name else [])

    def _body(*args):
        operands = list(args)
        if partition_name is not None:
            operands.append(partition_id_tensor())
        return tuple(
            _bass_exec_p.bind(
                *operands,
                out_avals=tuple(out_avals),
                in_names=tuple(all_in),
                out_names=tuple(out_names),
                lowering_input_output_aliases=(),
                sim_require_finite=True,
                sim_require_nnan=True,
                nc=nc,
            )
        )

    devices = jax.devices()[:n_cores]
    mesh = Mesh(np.asarray(devices), ("core",))
    n_outs = len(out_names)
    fn = jax.jit(
        shard_map(
            _body,
            mesh=mesh,
            in_specs=(PartitionSpec("core"),) * (n_params + n_outs),
            out_specs=(PartitionSpec("core"),) * n_outs,
            check_rep=False,
        ),
        donate_argnums=tuple(range(n_params, n_params + n_outs)),
        keep_unused=True,
    )
    return fn, in_names, out_names, out_avals


def _get_runner():
    if "runner" not in _CACHE:
        _CACHE["runner"] = _make_runner(_build_nc(), NCORES)
    return _CACHE["runner"]


def _prep_z(Z, zdt=None):
    """[n, d, m] f32/f16 -> per-core [P, NI, M, KJ] contiguous, stacked
    along axis 0 as [NCORES*P, NI, M, KJ] for the sharded runner."""
    import ml_dtypes

    zdt = ZDT if zdt is None else zdt
    npdt = np.float16 if zdt == "f16" else ml_dtypes.float8_e4m3
    Zr = np.asarray(Z).reshape(NCORES, NI, P, KJ, M)
    Zt = Zr.transpose(0, 2, 1, 4, 3)        # [c, P, NI, M, KJ]
    return np.ascontiguousarray(Zt.astype(npdt)).reshape(
        NCORES * P, NI, M, KJ
    )




def _prep_s8(s_f32, y):
    """mask + fp8 + per-core [P, NI*KJ] p-outer stack -> [NCORES*P, NI*KJ]."""
    import ml_dtypes

    rows = np.arange(N)
    s8 = s_f32.astype(ml_dtypes.float8_e4m3)
    s8[rows, np.asarray(y)] = -240.0
    sr = s8.reshape(NCORES, NI, P, KJ).transpose(0, 2, 1, 3)
    return np.ascontiguousarray(sr).reshape(NCORES * P, NI * KJ)


def kernel(s: np.ndarray, y: np.ndarray, Z: np.ndarray) -> np.ndarray:
    s = np.ascontiguousarray(s, dtype=np.float32)
    y = np.asarray(y)
    rows = np.arange(N)
    s_y = s[rows, y]
    if SDT == "f16":
        s_in = s.astype(np.float16)
        s_in[rows, y] = NEG16
    else:
        s_in = _prep_s8(s, y)
    zl = _prep_z(Z)

    import ml_dtypes as _mld

    arrays = {
        "s": s_in,
        "z": zl,
        "ident": np.tile(np.eye(128, dtype=np.float16), (NCORES, 1)),
        "ident8": np.tile(
            np.eye(128).astype(_mld.float8_e4m3), (NCORES, 1)
        ),
    }
    fn, in_names, out_names, out_avals = _get_runner()
    args = [arrays[n] for n in in_names]
    zeros = [
        np.zeros((NCORES * av.shape[0], *av.shape[1:]), av.dtype)
        for av in out_avals
    ]
    outs = fn(*args, *zeros)
    kth = np.asarray(outs[out_names.index("out")], dtype=np.float32)  # (N, M)
    kth_smooth = kth.mean(axis=1, dtype=np.float64)
    loss = np.maximum(1.0 + kth_smooth - s_y.astype(np.float64), 0.0)
    return np.float32(loss.mean())


def measure_hw_time(s, y, Z, reps_list=(16, 256), iters=12, build_kwargs=None):
    import time

    import jax

    build_kwargs = build_kwargs or {}
    s = np.ascontiguousarray(s, dtype=np.float32)
    rows = np.arange(N)
    sdt = build_kwargs.get("sdt") or SDT
    if sdt == "f16":
        s_masked = s.astype(np.float16)
        s_masked[rows, np.asarray(y)] = NEG16
    else:
        s_masked = _prep_s8(s, np.asarray(y))
    zl = _prep_z(Z, zdt=build_kwargs.get("zdt"))
    ident = np.eye(128, dtype=np.float16)
    import ml_dtypes as _mld

    ident8 = np.eye(128).astype(_mld.float8_e4m3)
    in_maps = [
        {
            "s": (s_masked[c * NI : (c + 1) * NI] if sdt == "f16"
                  else s_masked[c * P : (c + 1) * P]),
            "z": zl[c * P : (c + 1) * P],
            "ident": ident,
            "ident8": ident8,
        }
        for c in range(NCORES)
    ]
    results = {}
    for reps in reps_list:
        nc = _build_nc(loop_reps=reps, **build_kwargs)
        fn, in_names, out_names, out_avals = _make_runner(nc, NCORES)
        concat_in = [
            np.concatenate([np.asarray(m[name]) for m in in_maps], axis=0)
            for name in in_names
        ]
        dev_in = [jax.device_put(x) for x in concat_in]
        jax.block_until_ready(dev_in)
        times = []
        for _ in range(iters):
            zeros = [
                jax.device_put(
                    np.zeros((NCORES * av.shape[0], *av.shape[1:]), av.dtype)
                )
                for av in out_avals
            ]
            jax.block_until_ready(zeros)
            t0 = time.perf_counter()
            out = fn(*dev_in, *zeros)
            jax.block_until_ready(out)
            times.append(time.perf_counter() - t0)
        body = sorted(times[1:])
        results[reps] = body[len(body) // 2]
    ks = sorted(results)
    est_ns = None
    if len(ks) >= 2:
        est_ns = (results[ks[-1]] - results[ks[0]]) / (ks[-1] - ks[0]) * 1e9
    return est_ns, results
